# revision 1
# baseline (speedup 1.0000x reference)
"""CWT (continuous wavelet transform, pywt 'morl', 5 scales) as a Bass/Tile
kernel for 8 Trainium2 NeuronCores.

Math: for each scale s with integrated-wavelet filter k (length L), the
reference computes  trim(diff(full_corr(x, k))) * (-sqrt(s)) along T.  That
whole pipeline is a single correlation with the fixed kernel
    G[j] = sqrt(s) * (k[j] - k[j-1]),  j = 0..L  (k[-1] = k[L] = 0)
applied with offset  off = floor((L-2)/2) - (L-1):
    y[t] = sum_j x[t + off + j] * G[j]   (x zero-padded outside [0,T))
i.e. y = A_s @ x with the Toeplitz band matrix A_s[t, u] = G[u - t - off].

Kernel strategy (SPMD over 8 cores): core c owns t_out block I = c//2
(512 rows) and B-half h = c%2 (512 of 1024 batch*channel columns).

Large scales (167, 336: band ≥ T) run dense:
    psum[b, s] += X_chunk[q][:, b*128:+128].T @ wbuf[s][:, 1920-128q : +512]
over all 16 t_in chunks; wbuf is a [128 x 2432] sliding-window Toeplitz band
whose per-core t_out shift is baked into its *data* by the host, keeping one
static instruction stream for all cores.

Small scales (1, 27, 76: narrow band) are banded: only nb_s = 6/8/14 t_in
chunks touch a 512-row t_out block.  The host ships each core a shifted
window `xwin` of those 14 chunks (zero-padded at the edges) plus
core-independent mini band buffers, so the banded chains are static too:
    psum[b, s] += xwin_chunk[d0_s+5+j].T @ miniw[s][:, (nb_s-1-j)*128 : +512]

All matmuls are float32r (full-rate fp32 on the PE, ~1.4e-4 rel err),
N=512, accumulating in PSUM.
"""
import sys
import os

sys.path.insert(0, "/opt/trn_rl_repo")

import numpy as np

# ----------------------------------------------------------------- constants
WIDTHS = [1, 27, 76, 167, 336]
T = 2048
B = 1024  # 16 batch * 64 channels
N_CORES = 8
TOUT_PER_CORE = 512  # t_out rows per core (block I = core//2)
BH = 512  # B columns per core (half h = core%2)
NQ = T // 128  # 16 t_in chunks
NBH = BH // 128  # 4 column blocks per core
WBUF_W = 1920 + TOUT_PER_CORE  # dense band buffer width per large scale
BIG = [3, 4]  # scale indices processed densely
SMALL = [0, 1, 2]  # banded scale indices
XWIN_LO = -5  # xwin covers absolute chunks 4*blk + XWIN_LO .. +8
XWIN_NCH = 14

LAST_EXEC_NS = None  # set when CWT_TRACE=1


def _filters():
    """pywt 'morl' integrated wavelet, resampled per scale (matches reference)."""
    precision = 10
    n = 2**precision
    lb, ub = -8.0, 8.0
    t = np.linspace(lb, ub, n)
    psi = np.exp(-(t**2) / 2.0) * np.cos(5.0 * t)
    step = t[1] - t[0]
    int_psi = np.cumsum(psi) * step
    filts = []
    for scale in WIDTHS:
        j = (np.arange(scale * (ub - lb) + 1) / (scale * step)).astype(np.int64)
        j = j[j < n]
        filts.append(int_psi[j].astype(np.float32))
    return filts


def _g_kernels():
    """Effective correlation kernels G_s (len L+1) and offsets off_s."""
    gs = []
    for s, k in zip(WIDTHS, _filters()):
        k64 = k.astype(np.float64)
        L = len(k64)
        G = (np.sqrt(s) * np.diff(np.concatenate([[0.0], k64, [0.0]]))).astype(
            np.float32
        )
        off = int(np.floor((L - 2) / 2.0)) - (L - 1)
        gs.append((G, off))
    return gs


def _small_params():
    """(scale_idx, d0, nb, C, W) per banded scale."""
    gs = _g_kernels()
    out = []
    for si in SMALL:
        G, off = gs[si]
        L1 = len(G)
        d0 = off // 128
        span_hi = 511 + off + (L1 - 1) - 128 * d0
        nb = -(-(span_hi + 1) // 128)
        C = 128 * d0 + 128 * (nb - 1) - off
        W = 512 + 128 * (nb - 1)
        out.append((si, d0, nb, C, W))
    return out


def _toeplitz(G, C, W):
    p = np.arange(128)[:, None]
    w = np.arange(W)[None, :]
    idx = p - w + C
    valid = (idx >= 0) & (idx < len(G))
    return np.where(valid, G[np.clip(idx, 0, len(G) - 1)], np.float32(0.0)).astype(
        np.float32
    )


def _make_wbufs():
    """Per-t_out-block dense band buffers for the large scales."""
    gs = _g_kernels()
    bufs = []
    for blk in range(T // TOUT_PER_CORE):
        rc = blk * TOUT_PER_CORE
        per_scale = []
        for si in BIG:
            G, off = gs[si]
            per_scale.append(_toeplitz(G, 1920 - rc - off, WBUF_W))
        buf = np.stack(per_scale, axis=1).reshape(128, len(BIG) * WBUF_W)
        bufs.append(np.ascontiguousarray(buf))
    return bufs


def _make_miniw():
    """Core-independent banded buffers for the small scales, concatenated."""
    gs = _g_kernels()
    parts = [_toeplitz(gs[si][0], C, W) for si, d0, nb, C, W in _small_params()]
    return np.ascontiguousarray(np.concatenate(parts, axis=1))


_CONST_CACHE = None


def _consts():
    global _CONST_CACHE
    if _CONST_CACHE is None:
        _CONST_CACHE = (_make_wbufs(), _make_miniw(), _small_params())
    return _CONST_CACHE


# ----------------------------------------------------------------- program
_NC_CACHE = None


def _build_program():
    import concourse.bass as bass
    import concourse.bacc as bacc
    import concourse.mybir as mybir
    import concourse.tile as tile

    sp = _small_params()
    mw_w = sum(W for _, _, _, _, W in sp)

    nc = bacc.Bacc(None, target_bir_lowering=False, debug=False)

    x_d = nc.declare_dram_parameter("x", [T, BH], mybir.dt.float32r, isOutput=False)
    xw_d = nc.declare_dram_parameter(
        "xwin", [XWIN_NCH * 128, BH], mybir.dt.float32r, isOutput=False
    )
    w_d = nc.declare_dram_parameter(
        "wbuf", [128, len(BIG) * WBUF_W], mybir.dt.float32r, isOutput=False
    )
    mw_d = nc.declare_dram_parameter(
        "miniw", [128, mw_w], mybir.dt.float32r, isOutput=False
    )
    out_d = nc.declare_dram_parameter(
        "out", [5, 128, NBH * TOUT_PER_CORE], mybir.dt.float32, isOutput=True
    )

    with tile.TileContext(nc) as tc:
        with (
            tc.tile_pool(name="xp", bufs=1) as xp,
            tc.tile_pool(name="wp", bufs=1) as wp,
            tc.tile_pool(name="op", bufs=1) as op,
            tc.tile_pool(name="pp", bufs=1, space=bass.MemorySpace.PSUM) as pp,
        ):
            # band buffers ride the ACT HWDGE ring so the x/xwin stream on
            # the SP ring doesn't queue behind them
            # ~0.65us issue per dma_start and ~165GB/s per queue: keep
            # pieces around 0.5-1MB so transfers spread across queues
            wsb = wp.tile([128, len(BIG) * WBUF_W], mybir.dt.float32r, tag="wsb")
            wh = WBUF_W // 2
            for i in range(len(BIG)):
                for lo, hi in ((0, wh), (wh, WBUF_W)):
                    nc.scalar.dma_start(
                        wsb[:, i * WBUF_W + lo : i * WBUF_W + hi],
                        w_d[:, i * WBUF_W + lo : i * WBUF_W + hi],
                    )
            mwsb = wp.tile([128, mw_w], mybir.dt.float32r, tag="mwsb")
            mh = mw_w // 2
            nc.scalar.dma_start(mwsb[:, :mh], mw_d[:, :mh])
            nc.scalar.dma_start(mwsb[:, mh:], mw_d[:, mh:])

            # x chunks: grouped DMAs (each dma_start costs ~650ns of HWDGE
            # issue; Tile dependency tracking is AP-range based so consumers
            # wait only for their group)
            xsb = xp.tile([128, NQ * BH], mybir.dt.float32r, tag="xsb", name="xsb")
            for g0, g1 in ((0, 1), (1, 2), (2, 4), (4, 6), (6, 8), (8, 10), (10, 12), (12, 14), (14, NQ)):
                nc.sync.dma_start(
                    xsb[:, g0 * BH : g1 * BH].rearrange("p (q b) -> p q b", b=BH),
                    x_d[g0 * 128 : g1 * 128, :].rearrange("(q p) b -> p q b", p=128),
                )
            xwsb = xp.tile(
                [128, XWIN_NCH * BH], mybir.dt.float32r, tag="xwsb", name="xwsb"
            )
            for g0, g1 in ((0, 4), (4, 7), (7, 11), (11, XWIN_NCH)):
                nc.sync.dma_start(
                    xwsb[:, g0 * BH : g1 * BH].rearrange("p (q b) -> p q b", b=BH),
                    xw_d[g0 * 128 : g1 * 128, :].rearrange("(q p) b -> p q b", p=128),
                )

            grp = 0

            def run_chain(s, ps, mk_ops):
                # q-major interleave across the 4 b-chains: one arriving
                # x-chunk feeds 4 matmuls, keeping the PE arrival-paced
                # instead of stalling on a single chain's last chunk
                nonlocal grp
                stg = stgs[s]
                psums = []
                opl = []
                for b in range(NBH):
                    psums.append(
                        pp.tile(
                            [128, TOUT_PER_CORE],
                            mybir.dt.float32,
                            tag=f"ps{(grp % 2) * 4 + b}",
                            name=f"ps_{s}_{b}",
                        )
                    )
                    opl.append(mk_ops(b))
                nq = len(opl[0])
                for i in range(nq):
                    for b in range(NBH):
                        lhsT, rhs = opl[b][i]
                        nc.tensor.matmul(
                            psums[b][:],
                            lhsT,
                            rhs,
                            start=(i == 0),
                            stop=(i == nq - 1),
                        )
                for b in range(NBH):
                    nc.vector.tensor_copy(
                        stg[:, b * TOUT_PER_CORE : (b + 1) * TOUT_PER_CORE],
                        psums[b][:],
                    )
                grp += 1

            stgs = {}
            for s in BIG + SMALL:
                stgs[s] = op.tile(
                    [128, NBH * TOUT_PER_CORE],
                    mybir.dt.float32,
                    tag=f"stg{s}",
                    name=f"stg{s}",
                )

            # dense large scales first (need only x + their wbuf region)
            for i, s in enumerate(BIG):
                def mk_dense(b, i=i):
                    return [
                        (
                            xsb[:, q * BH + b * 128 : q * BH + (b + 1) * 128],
                            wsb[
                                :,
                                i * WBUF_W
                                + 1920
                                - 128 * q : i * WBUF_W
                                + 1920
                                - 128 * q
                                + TOUT_PER_CORE,
                            ],
                        )
                        for q in range(NQ)
                    ]

                run_chain(s, None, mk_dense)
                nc.scalar.dma_start(out_d[s], stgs[s][:])

            # banded small scales (need xwin + miniw)
            mw_base = 0
            for si, d0, nb, C, W in sp:
                def mk_small(b, d0=d0, nb=nb, base=mw_base):
                    ops = []
                    for j in range(nb):
                        cj = d0 - XWIN_LO + j
                        ops.append(
                            (
                                xwsb[:, cj * BH + b * 128 : cj * BH + (b + 1) * 128],
                                mwsb[
                                    :,
                                    base
                                    + (nb - 1 - j) * 128 : base
                                    + (nb - 1 - j) * 128
                                    + TOUT_PER_CORE,
                                ],
                            )
                        )
                    return ops

                run_chain(si, None, mk_small)
                if si == SMALL[-1]:
                    half = NBH * TOUT_PER_CORE // 2
                    nc.scalar.dma_start(out_d[si][:, :half], stgs[si][:, :half])
                    nc.scalar.dma_start(out_d[si][:, half:], stgs[si][:, half:])
                else:
                    nc.scalar.dma_start(out_d[si], stgs[si][:])
                mw_base += W

    nc.compile()  # bacc passes legalize multi-sem-waits for walrus codegen
    return nc


def _program():
    global _NC_CACHE
    if _NC_CACHE is None:
        _NC_CACHE = _build_program()
    return _NC_CACHE


# ----------------------------------------------------------------- entry
def kernel(x: np.ndarray) -> np.ndarray:
    """x: [16, 2048, 64] float32 -> [16, 2048, 64, 5] float32"""
    global LAST_EXEC_NS
    from concourse.bass_utils import run_bass_kernel_spmd

    n, t, c = x.shape
    assert (t, n * c) == (T, B), (x.shape,)

    X = np.ascontiguousarray(x.transpose(1, 0, 2).reshape(T, B).astype(np.float32))
    wbufs, miniw, sp = _consts()
    in_maps = []
    for core in range(N_CORES):
        blk, h = core // 2, core % 2
        xh = X[:, h * BH : (h + 1) * BH]
        lo = (4 * blk + XWIN_LO) * 128
        xwin = np.zeros((XWIN_NCH * 128, BH), np.float32)
        a = max(0, lo)
        bnd = min(T, lo + XWIN_NCH * 128)
        if bnd > a:
            xwin[a - lo : bnd - lo, :] = xh[a:bnd, :]
        in_maps.append(
            {
                "x": np.ascontiguousarray(xh),
                "xwin": xwin,
                "wbuf": wbufs[blk],
                "miniw": miniw,
            }
        )

    nc = _program()
    trace = bool(int(os.environ.get("CWT_TRACE", "0")))
    res = run_bass_kernel_spmd(nc, in_maps, list(range(N_CORES)), trace=trace)
    if trace:
        LAST_EXEC_NS = res.exec_time_ns
        globals()["LAST_RESULTS"] = res

    # per-core out: [5, 128, NBH*512] -> assemble [5, T, B]
    Y = np.empty((5, T, B), np.float32)
    for core in range(N_CORES):
        blk, h = core // 2, core % 2
        o = res.results[core]["out"].reshape(5, 128, NBH, TOUT_PER_CORE)
        # [s, p, b, n] -> Y[s, 512*blk + n, h*512 + 128b + p]
        Y[
            :, blk * TOUT_PER_CORE : (blk + 1) * TOUT_PER_CORE,
            h * BH : (h + 1) * BH,
        ] = o.transpose(0, 3, 2, 1).reshape(5, TOUT_PER_CORE, BH)
    return np.ascontiguousarray(
        Y.reshape(5, T, n, c).transpose(2, 1, 3, 0).astype(np.float32)
    )



# revision 2
# speedup vs baseline: 1.2382x; 1.2382x over previous
"""CWT (continuous wavelet transform, pywt 'morl', 5 scales) as a Bass/Tile
kernel for 8 Trainium2 NeuronCores.

Math: for each scale s with integrated-wavelet filter k (length L), the
reference computes  trim(diff(full_corr(x, k))) * (-sqrt(s)) along T.  That
whole pipeline is a single correlation with the fixed kernel
    G[j] = sqrt(s) * (k[j] - k[j-1]),  j = 0..L  (k[-1] = k[L] = 0)
applied with offset  off = floor((L-2)/2) - (L-1):
    y[t] = sum_j x[t + off + j] * G[j]   (x zero-padded outside [0,T))
i.e. y = A_s @ x with the Toeplitz band matrix A_s[t, u] = G[u - t - off].

Kernel strategy v2 (SPMD over 8 cores): pure B-sharding.  Core c owns the
128 batch*channel columns [128c, 128c+128); every core computes all 2048
t_out rows for its columns.  All t_out-block indices are then
core-independent, so a single static instruction stream works with NO
per-core shifted data: the banded scales read x chunks directly (chunks
outside [0,16) are the zero-padding and are simply dropped).

All matmul operands are bfloat16 (full PE rate, half the DMA bytes of
fp32r; ~2.4e-3 rel err vs the 2e-2 budget).  Per (scale, t_out block I):
    psum[b, tau] += X_chunk[q].T @ Wsc[:, w0(s,I,q) : +512]
accumulated over the chunks q that intersect the band, where Wsc is a
per-scale Toeplitz sliding window (width max 128q-512I spread + 512),
identical on every core.  220 total matmuls/core (vs 240 in v1 — edge
chunks whose band falls outside [0,T) are skipped).

DMA: x (0.5MB) + W (3.0MB) in bf16, consumption-ordered pieces split
across the two HWDGE rings (sync=x+most outs, scalar=W+late outs); each
(s, I) output piece is copied psum->SBUF and DMA'd out as soon as its
chain stops.  The last chain is scale 1 / block 3 (5 matmuls) and its
copy+store is split across vector+scalar engines and both rings to
minimize the tail.
"""
import sys
import os

sys.path.insert(0, "/opt/trn_rl_repo")

import numpy as np

# ----------------------------------------------------------------- constants
WIDTHS = [1, 27, 76, 167, 336]
T = 2048
B = 1024  # 16 batch * 64 channels
N_CORES = 8
BPC = B // N_CORES  # 128 batch*channel columns per core
NQ = T // 128  # 16 t_in chunks
NI = T // 512  # 4 t_out blocks per core (all computed by every core)
NSC = len(WIDTHS)

LAST_EXEC_NS = None  # set when CWT_TRACE=1


def _filters():
    """pywt 'morl' integrated wavelet, resampled per scale (matches reference)."""
    precision = 10
    n = 2**precision
    lb, ub = -8.0, 8.0
    t = np.linspace(lb, ub, n)
    psi = np.exp(-(t**2) / 2.0) * np.cos(5.0 * t)
    step = t[1] - t[0]
    int_psi = np.cumsum(psi) * step
    filts = []
    for scale in WIDTHS:
        j = (np.arange(scale * (ub - lb) + 1) / (scale * step)).astype(np.int64)
        j = j[j < n]
        filts.append(int_psi[j].astype(np.float32))
    return filts


def _g_kernels():
    """Effective correlation kernels G_s (len L+1) and offsets off_s."""
    gs = []
    for s, k in zip(WIDTHS, _filters()):
        k64 = k.astype(np.float64)
        L = len(k64)
        G = (np.sqrt(s) * np.diff(np.concatenate([[0.0], k64, [0.0]]))).astype(
            np.float32
        )
        off = int(np.floor((L - 2) / 2.0)) - (L - 1)
        gs.append((G, off))
    return gs


def _plan():
    """Per-scale Toeplitz window geometry + per-block chunk ranges.

    w0(s, I, q) = C_s + off_s - (128q - 512I) is the first W column of the
    512-wide rhs slice for chunk q of t_out block I.
    """
    plans = []
    for G, off in _g_kernels():
        L1 = len(G)
        qr = []
        vs = []
        for I in range(NI):
            lo = max(0, (512 * I + off) // 128)
            hi = min(NQ - 1, (512 * I + 511 + off + L1 - 1) // 128)
            qr.append((lo, hi))
            vs += [128 * q - 512 * I for q in range(lo, hi + 1)]
        C = max(vs) - off
        W = max(vs) - min(vs) + 512
        plans.append({"off": off, "L1": L1, "qr": qr, "C": C, "W": W, "G": G})
    return plans


def _toeplitz(G, C, W):
    p = np.arange(128)[:, None]
    w = np.arange(W)[None, :]
    idx = p - w + C
    valid = (idx >= 0) & (idx < len(G))
    return np.where(valid, G[np.clip(idx, 0, len(G) - 1)], np.float32(0.0)).astype(
        np.float32
    )


# chain processing groups: (scale, t_out block).  Scale 1's short chains
# bracket the schedule: two start it (smallest W piece -> earliest first
# matmul) and two end it (shortest chains -> smallest output tail).
GROUPS = [
    [(0, 0), (0, 1)],
    [(1, 0), (1, 1), (1, 2), (1, 3)],
    [(2, 0), (2, 1), (2, 2), (2, 3)],
    [(3, 0), (3, 1), (3, 2), (3, 3)],
    [(4, 0), (4, 1), (4, 2), (4, 3)],
    [(0, 2), (0, 3)],
]


def _schedule(plans):
    """Emission-ordered list of (s, I, q, start, stop, w0)."""
    sched = []
    for chains in GROUPS:
        qmin = min(plans[s]["qr"][I][0] for s, I in chains)
        qmax = max(plans[s]["qr"][I][1] for s, I in chains)
        for q in range(qmin, qmax + 1):
            for s, I in chains:
                lo, hi = plans[s]["qr"][I]
                if lo <= q <= hi:
                    w0 = plans[s]["C"] + plans[s]["off"] - (128 * q - 512 * I)
                    sched.append((s, I, q, q == lo, q == hi, w0))
    return sched


_CONST_CACHE = None


def _consts():
    global _CONST_CACHE
    if _CONST_CACHE is None:
        import ml_dtypes

        plans = _plan()
        wbuf = np.concatenate(
            [_toeplitz(p["G"], p["C"], p["W"]) for p in plans], axis=1
        ).astype(ml_dtypes.bfloat16)
        _CONST_CACHE = (plans, np.ascontiguousarray(wbuf))
    return _CONST_CACHE


# ----------------------------------------------------------------- program
_NC_CACHE = None


def _w_pieces(plans, sched):
    """Split the concatenated W buffer into DMA pieces, ordered by first use.

    Scale 0 is split at 512-col granularity so the very first piece is tiny;
    the rest use ~1024-col (256KB bf16) pieces.
    """
    bases = []
    b = 0
    for p in plans:
        bases.append(b)
        b += p["W"]
    pieces = []  # (first_use_idx, lo, hi) in concat cols
    for s, p in enumerate(plans):
        step = 512 if s == 0 else 1024
        cuts = list(range(0, p["W"], step)) + [p["W"]]
        for lo, hi in zip(cuts[:-1], cuts[1:]):
            first = None
            for i, (ss, I, q, st, sp, w0) in enumerate(sched):
                if ss == s and w0 < hi and w0 + 512 > lo:
                    first = i
                    break
            pieces.append((first if first is not None else len(sched), bases[s] + lo, bases[s] + hi))
    pieces.sort()
    return bases, [(lo, hi) for _, lo, hi in pieces]


def _build_program():
    import concourse.bass as bass
    import concourse.bacc as bacc
    import concourse.mybir as mybir
    import concourse.tile as tile

    plans, _ = _consts()
    sched = _schedule(plans)
    wtot = sum(p["W"] for p in plans)
    bases, wpieces = _w_pieces(plans, sched)

    nc = bacc.Bacc(None, target_bir_lowering=False, debug=False)

    x_d = nc.declare_dram_parameter("x", [128, NQ * BPC], mybir.dt.bfloat16, isOutput=False)
    w_d = nc.declare_dram_parameter("w", [128, wtot], mybir.dt.bfloat16, isOutput=False)
    out_d = nc.declare_dram_parameter(
        "out", [NSC, 128, T], mybir.dt.float32, isOutput=True
    )

    # which ring carries each scale's output pieces (sync carries x early,
    # scalar carries W early; both are free by the time their outs flow)
    OUT_ENG = {0: "sync", 1: "sync", 2: "sync", 3: "sync", 4: "scalar"}
    LAST_CHAIN = (0, 3)

    with tile.TileContext(nc) as tc:
        with (
            tc.tile_pool(name="xp", bufs=1) as xp,
            tc.tile_pool(name="wp", bufs=1) as wp,
            tc.tile_pool(name="op", bufs=1) as op,
            tc.tile_pool(name="pp", bufs=1, space=bass.MemorySpace.PSUM) as pp,
        ):
            # x chunks on the sync (SP) HWDGE ring, 4-chunk pieces in
            # consumption order
            xsb = xp.tile([128, NQ * BPC], mybir.dt.bfloat16, tag="xsb", name="xsb")
            for g0 in range(0, NQ, 4):
                nc.sync.dma_start(
                    xsb[:, g0 * BPC : (g0 + 4) * BPC],
                    x_d[:, g0 * BPC : (g0 + 4) * BPC],
                )

            # W pieces on the scalar (ACT) ring, first-use order
            wsb = wp.tile([128, wtot], mybir.dt.bfloat16, tag="wsb", name="wsb")
            for lo, hi in wpieces:
                nc.scalar.dma_start(wsb[:, lo:hi], w_d[:, lo:hi])

            stgs = [
                op.tile([128, T], mybir.dt.float32, tag=f"stg{s}", name=f"stg{s}")
                for s in range(NSC)
            ]

            # psum tags: groups alternate the two 4-bank halves
            psums = {}
            for gi, chains in enumerate(GROUPS):
                for ci, (s, I) in enumerate(chains):
                    psums[(s, I)] = pp.tile(
                        [128, 512],
                        mybir.dt.float32,
                        tag=f"ps{(gi % 2) * 4 + ci}",
                        name=f"ps_{s}_{I}",
                    )

            for s, I, q, start, stop, w0 in sched:
                nc.tensor.matmul(
                    psums[(s, I)][:],
                    xsb[:, q * BPC : (q + 1) * BPC],
                    wsb[:, bases[s] + w0 : bases[s] + w0 + 512],
                    start=start,
                    stop=stop,
                )
                if stop:
                    stg = stgs[s]
                    c0 = 512 * I
                    if (s, I) == LAST_CHAIN:
                        # split the final piece across engines + rings to
                        # shorten the tail
                        nc.vector.tensor_copy(
                            stg[:, c0 : c0 + 256], psums[(s, I)][:, 0:256]
                        )
                        nc.scalar.copy(
                            stg[:, c0 + 256 : c0 + 512], psums[(s, I)][:, 256:512]
                        )
                        nc.sync.dma_start(
                            out_d[s][:, c0 : c0 + 256], stg[:, c0 : c0 + 256]
                        )
                        nc.scalar.dma_start(
                            out_d[s][:, c0 + 256 : c0 + 512],
                            stg[:, c0 + 256 : c0 + 512],
                        )
                    else:
                        nc.vector.tensor_copy(
                            stg[:, c0 : c0 + 512], psums[(s, I)][:]
                        )
                        eng = nc.sync if OUT_ENG[s] == "sync" else nc.scalar
                        eng.dma_start(
                            out_d[s][:, c0 : c0 + 512], stg[:, c0 : c0 + 512]
                        )

    nc.compile()
    return nc


def _program():
    global _NC_CACHE
    if _NC_CACHE is None:
        _NC_CACHE = _build_program()
    return _NC_CACHE


# ----------------------------------------------------------------- entry
def kernel(x: np.ndarray) -> np.ndarray:
    """x: [16, 2048, 64] float32 -> [16, 2048, 64, 5] float32"""
    global LAST_EXEC_NS
    import ml_dtypes
    from concourse.bass_utils import run_bass_kernel_spmd

    n, t, c = x.shape
    assert (t, n * c) == (T, B), (x.shape,)

    X = x.transpose(1, 0, 2).reshape(T, B).astype(np.float32)
    _, wbuf = _consts()
    in_maps = []
    for core in range(N_CORES):
        xc = X[:, core * BPC : (core + 1) * BPC]  # [2048, 128]
        xc = (
            xc.reshape(NQ, 128, BPC)
            .transpose(1, 0, 2)
            .reshape(128, NQ * BPC)
            .astype(ml_dtypes.bfloat16)
        )
        in_maps.append({"x": np.ascontiguousarray(xc), "w": wbuf})

    nc = _program()
    trace = bool(int(os.environ.get("CWT_TRACE", "0")))
    res = run_bass_kernel_spmd(nc, in_maps, list(range(N_CORES)), trace=trace)
    if trace:
        LAST_EXEC_NS = res.exec_time_ns
        globals()["LAST_RESULTS"] = res

    # per-core out: [5, 128, 2048] (b-local, t) -> Y [5, T, B]
    Y = np.empty((NSC, T, B), np.float32)
    for core in range(N_CORES):
        o = res.results[core]["out"]  # [5, 128, 2048]
        Y[:, :, core * BPC : (core + 1) * BPC] = o.transpose(0, 2, 1)
    return np.ascontiguousarray(
        Y.reshape(NSC, T, n, c).transpose(2, 1, 3, 0).astype(np.float32)
    )


# revision 6
# speedup vs baseline: 1.3239x; 1.0692x over previous
"""CWT (continuous wavelet transform, pywt 'morl', 5 scales) as a Bass/Tile
kernel for 8 Trainium2 NeuronCores.

Math: for each scale s with integrated-wavelet filter k (length L), the
reference computes  trim(diff(full_corr(x, k))) * (-sqrt(s)) along T.  That
whole pipeline is a single correlation with the fixed kernel
    G[j] = sqrt(s) * (k[j] - k[j-1]),  j = 0..L  (k[-1] = k[L] = 0)
applied with offset  off = floor((L-2)/2) - (L-1):
    y[t] = sum_j x[t + off + j] * G[j]   (x zero-padded outside [0,T))
i.e. y = A_s @ x with the Toeplitz band matrix A_s[t, u] = G[u - t - off].

Kernel strategy v2 (SPMD over 8 cores): pure B-sharding.  Core c owns the
128 batch*channel columns [128c, 128c+128); every core computes all 2048
t_out rows for its columns.  All t_out-block indices are then
core-independent, so a single static instruction stream works with NO
per-core shifted data: the banded scales read x chunks directly (chunks
outside [0,16) are the zero-padding and are simply dropped).

All matmul operands are bfloat16 (full PE rate, half the DMA bytes of
fp32r; ~2.4e-3 rel err vs the 2e-2 budget).  Per (scale, t_out block I):
    psum[b, tau] += X_chunk[q].T @ Wsc[:, w0(s,I,q) : +512]
accumulated over the chunks q that intersect the band, where Wsc is a
per-scale Toeplitz sliding window (width max 128q-512I spread + 512),
identical on every core.  220 total matmuls/core (vs 240 in v1 — edge
chunks whose band falls outside [0,T) are skipped).

DMA: x (0.5MB) + W (3.0MB) in bf16, consumption-ordered pieces split
across the two HWDGE rings (sync=x+most outs, scalar=W+late outs); each
(s, I) output piece is copied psum->SBUF and DMA'd out as soon as its
chain stops.  The last chain is scale 1 / block 3 (5 matmuls) and its
copy+store is split across vector+scalar engines and both rings to
minimize the tail.
"""
import sys
import os

sys.path.insert(0, "/opt/trn_rl_repo")

import numpy as np

# ----------------------------------------------------------------- constants
WIDTHS = [1, 27, 76, 167, 336]
T = 2048
B = 1024  # 16 batch * 64 channels
N_CORES = 8
BPC = B // N_CORES  # 128 batch*channel columns per core
NQ = T // 128  # 16 t_in chunks
NI = T // 512  # 4 t_out blocks per core (all computed by every core)
NSC = len(WIDTHS)

LAST_EXEC_NS = None  # set when CWT_TRACE=1


def _filters():
    """pywt 'morl' integrated wavelet, resampled per scale (matches reference)."""
    precision = 10
    n = 2**precision
    lb, ub = -8.0, 8.0
    t = np.linspace(lb, ub, n)
    psi = np.exp(-(t**2) / 2.0) * np.cos(5.0 * t)
    step = t[1] - t[0]
    int_psi = np.cumsum(psi) * step
    filts = []
    for scale in WIDTHS:
        j = (np.arange(scale * (ub - lb) + 1) / (scale * step)).astype(np.int64)
        j = j[j < n]
        filts.append(int_psi[j].astype(np.float32))
    return filts


def _g_kernels():
    """Effective correlation kernels G_s (len L+1) and offsets off_s."""
    gs = []
    for s, k in zip(WIDTHS, _filters()):
        k64 = k.astype(np.float64)
        L = len(k64)
        G = (np.sqrt(s) * np.diff(np.concatenate([[0.0], k64, [0.0]]))).astype(
            np.float32
        )
        off = int(np.floor((L - 2) / 2.0)) - (L - 1)
        gs.append((G, off))
    return gs


def _plan():
    """Per-scale Toeplitz window geometry + per-block chunk ranges.

    w0(s, I, q) = C_s + off_s - (128q - 512I) is the first W column of the
    512-wide rhs slice for chunk q of t_out block I.
    """
    plans = []
    for G, off in _g_kernels():
        L1 = len(G)
        qr = []
        vs = []
        for I in range(NI):
            lo = max(0, (512 * I + off) // 128)
            hi = min(NQ - 1, (512 * I + 511 + off + L1 - 1) // 128)
            qr.append((lo, hi))
            vs += [128 * q - 512 * I for q in range(lo, hi + 1)]
        C = max(vs) - off
        W = max(vs) - min(vs) + 512
        plans.append({"off": off, "L1": L1, "qr": qr, "C": C, "W": W, "G": G})
    return plans


def _toeplitz(G, C, W):
    p = np.arange(128)[:, None]
    w = np.arange(W)[None, :]
    idx = p - w + C
    valid = (idx >= 0) & (idx < len(G))
    return np.where(valid, G[np.clip(idx, 0, len(G) - 1)], np.float32(0.0)).astype(
        np.float32
    )


# chain processing groups: (scale, t_out block).  Scale 1's short chains
# bracket the schedule: two start it (smallest W piece -> earliest first
# matmul) and two end it (shortest chains -> smallest output tail).  The
# dense scales run mid-schedule so their 4-chains-end-together output
# bursts drain while later groups compute.
GROUPS = [
    [(0, 0), (0, 1)],
    [(1, 0), (1, 1), (1, 2), (1, 3)],
    [(3, 0), (3, 1), (3, 2), (3, 3)],
    [(4, 0), (4, 1), (4, 2), (4, 3)],
    [(2, 0), (2, 1), (2, 2), (2, 3)],
    [(0, 2), (0, 3)],
]


def _schedule(plans):
    """Emission-ordered list of (s, I, q, start, stop, w0)."""
    sched = []
    for chains in GROUPS:
        qmin = min(plans[s]["qr"][I][0] for s, I in chains)
        qmax = max(plans[s]["qr"][I][1] for s, I in chains)
        for q in range(qmin, qmax + 1):
            for s, I in chains:
                lo, hi = plans[s]["qr"][I]
                if lo <= q <= hi:
                    w0 = plans[s]["C"] + plans[s]["off"] - (128 * q - 512 * I)
                    sched.append((s, I, q, q == lo, q == hi, w0))
    return sched


_CONST_CACHE = None


def _consts():
    global _CONST_CACHE
    if _CONST_CACHE is None:
        import ml_dtypes

        plans = _plan()
        wbuf = np.concatenate(
            [_toeplitz(p["G"], p["C"], p["W"]) for p in plans], axis=1
        ).astype(ml_dtypes.bfloat16)
        _CONST_CACHE = (plans, np.ascontiguousarray(wbuf))
    return _CONST_CACHE


# ----------------------------------------------------------------- program
_NC_CACHE = None


def _w_pieces(plans, sched):
    """Split the concatenated W buffer into DMA pieces, ordered by first use.

    Scale 0's first piece is cut to exactly cover its first two schedule
    steps so the PE can start ~1us sooner; other scales use 2 halves
    (few pieces -> few DMA sems -> short semaphore-teardown epilogue).
    """
    bases = []
    b = 0
    for p in plans:
        bases.append(b)
        b += p["W"]
    pieces = []  # (first_use_idx, lo, hi) in concat cols
    for s, p in enumerate(plans):
        if s == 0:
            cuts = [0, 384, p["W"]]
        else:
            h = (p["W"] // 2 + 127) & ~127
            cuts = [0, h, p["W"]]
        for lo, hi in zip(cuts[:-1], cuts[1:]):
            first = None
            for i, (ss, I, q, st, sp, w0) in enumerate(sched):
                if ss == s and w0 < hi and w0 + 512 > lo:
                    first = i
                    break
            pieces.append((first if first is not None else len(sched), bases[s] + lo, bases[s] + hi))
    pieces.sort()
    return bases, [(lo, hi) for _, lo, hi in pieces]


def _build_program():
    import concourse.bass as bass
    import concourse.bacc as bacc
    import concourse.mybir as mybir
    import concourse.tile as tile

    plans, _ = _consts()
    sched = _schedule(plans)
    wtot = sum(p["W"] for p in plans)
    bases, wpieces = _w_pieces(plans, sched)

    nc = bacc.Bacc(None, target_bir_lowering=False, debug=False)

    x_d = nc.declare_dram_parameter("x", [128, NQ * BPC], mybir.dt.bfloat16, isOutput=False)
    w_d = nc.declare_dram_parameter("w", [128, wtot], mybir.dt.bfloat16, isOutput=False)
    # outputs leave as bf16 (host upcasts): halves the store bytes; adds
    # ~0.1% rms quantization vs the 2e-2 budget
    out_d = nc.declare_dram_parameter(
        "out", [NSC, 128, T], mybir.dt.bfloat16, isOutput=True
    )

    # ring for each whole-scale output store (sync carries x early, scalar
    # carries W early; both are free by the time these flow mid-schedule)
    OUT_ENG = {1: "sync", 3: "sync", 4: "scalar", 2: "scalar"}
    LAST_CHAIN = (0, 3)

    with tile.TileContext(nc) as tc:
        with (
            tc.tile_pool(name="xp", bufs=1) as xp,
            tc.tile_pool(name="wp", bufs=1) as wp,
            tc.tile_pool(name="op", bufs=1) as op,
            tc.tile_pool(name="pp", bufs=1, space=bass.MemorySpace.PSUM) as pp,
        ):
            # x chunks on the sync (SP) HWDGE ring in consumption order
            xsb = xp.tile([128, NQ * BPC], mybir.dt.bfloat16, tag="xsb", name="xsb")
            for g0, g1 in ((0, 6), (6, 11), (11, NQ)):
                nc.sync.dma_start(
                    xsb[:, g0 * BPC : g1 * BPC],
                    x_d[:, g0 * BPC : g1 * BPC],
                )

            # W pieces on the scalar (ACT) ring, first-use order
            wsb = wp.tile([128, wtot], mybir.dt.bfloat16, tag="wsb", name="wsb")
            for lo, hi in wpieces:
                nc.scalar.dma_start(wsb[:, lo:hi], w_d[:, lo:hi])

            stgs = [
                op.tile([128, T], mybir.dt.bfloat16, tag=f"stg{s}", name=f"stg{s}")
                for s in range(NSC)
            ]

            # psum tags: groups alternate the two 4-bank halves
            psums = {}
            for gi, chains in enumerate(GROUPS):
                for ci, (s, I) in enumerate(chains):
                    psums[(s, I)] = pp.tile(
                        [128, 512],
                        mybir.dt.float32,
                        tag=f"ps{(gi % 2) * 4 + ci}",
                        name=f"ps_{s}_{I}",
                    )

            done = {s: 0 for s in range(NSC)}
            for s, I, q, start, stop, w0 in sched:
                nc.tensor.matmul(
                    psums[(s, I)][:],
                    xsb[:, q * BPC : (q + 1) * BPC],
                    wsb[:, bases[s] + w0 : bases[s] + w0 + 512],
                    start=start,
                    stop=stop,
                )
                if not stop:
                    continue
                stg = stgs[s]
                c0 = 512 * I
                done[s] += 1
                if s == 0:
                    # scale 1 brackets the schedule; store per-block pieces
                    if (s, I) == LAST_CHAIN:
                        # split the final piece across engines + rings to
                        # shorten the tail
                        nc.vector.tensor_copy(
                            stg[:, c0 : c0 + 256], psums[(s, I)][:, 0:256]
                        )
                        nc.scalar.copy(
                            stg[:, c0 + 256 : c0 + 512], psums[(s, I)][:, 256:512]
                        )
                        nc.sync.dma_start(
                            out_d[s][:, c0 : c0 + 256], stg[:, c0 : c0 + 256]
                        )
                        nc.scalar.dma_start(
                            out_d[s][:, c0 + 256 : c0 + 512],
                            stg[:, c0 + 256 : c0 + 512],
                        )
                    else:
                        nc.vector.tensor_copy(
                            stg[:, c0 : c0 + 512], psums[(s, I)][:]
                        )
                        nc.sync.dma_start(
                            out_d[s][:, c0 : c0 + 512], stg[:, c0 : c0 + 512]
                        )
                else:
                    nc.vector.tensor_copy(stg[:, c0 : c0 + 512], psums[(s, I)][:])
                    if done[s] == NI:
                        # whole-scale store once the last block is staged
                        eng = nc.sync if OUT_ENG[s] == "sync" else nc.scalar
                        eng.dma_start(out_d[s], stg[:])

    nc.compile()
    return nc


def _program():
    global _NC_CACHE
    if _NC_CACHE is None:
        _NC_CACHE = _build_program()
    return _NC_CACHE


# ----------------------------------------------------------------- entry
def kernel(x: np.ndarray) -> np.ndarray:
    """x: [16, 2048, 64] float32 -> [16, 2048, 64, 5] float32"""
    global LAST_EXEC_NS
    import ml_dtypes
    from concourse.bass_utils import run_bass_kernel_spmd

    n, t, c = x.shape
    assert (t, n * c) == (T, B), (x.shape,)

    X = x.transpose(1, 0, 2).reshape(T, B).astype(np.float32)
    _, wbuf = _consts()
    in_maps = []
    for core in range(N_CORES):
        xc = X[:, core * BPC : (core + 1) * BPC]  # [2048, 128]
        xc = (
            xc.reshape(NQ, 128, BPC)
            .transpose(1, 0, 2)
            .reshape(128, NQ * BPC)
            .astype(ml_dtypes.bfloat16)
        )
        in_maps.append({"x": np.ascontiguousarray(xc), "w": wbuf})

    nc = _program()
    trace = bool(int(os.environ.get("CWT_TRACE", "0")))
    res = run_bass_kernel_spmd(nc, in_maps, list(range(N_CORES)), trace=trace)
    if trace:
        LAST_EXEC_NS = res.exec_time_ns
        globals()["LAST_RESULTS"] = res

    # per-core out: [5, 128, 2048] bf16 (b-local, t) -> Y [5, T, B] fp32
    Y = np.empty((NSC, T, B), np.float32)
    for core in range(N_CORES):
        o = np.asarray(res.results[core]["out"]).astype(np.float32)
        Y[:, :, core * BPC : (core + 1) * BPC] = o.transpose(0, 2, 1)
    return np.ascontiguousarray(
        Y.reshape(NSC, T, n, c).transpose(2, 1, 3, 0).astype(np.float32)
    )


# revision 13
# speedup vs baseline: 1.3259x; 1.0016x over previous
"""CWT (continuous wavelet transform, pywt 'morl', 5 scales) as a Bass/Tile
kernel for 8 Trainium2 NeuronCores.

Math: for each scale s with integrated-wavelet filter k (length L), the
reference computes  trim(diff(full_corr(x, k))) * (-sqrt(s)) along T.  That
whole pipeline is a single correlation with the fixed kernel
    G[j] = sqrt(s) * (k[j] - k[j-1]),  j = 0..L  (k[-1] = k[L] = 0)
applied with offset  off = floor((L-2)/2) - (L-1):
    y[t] = sum_j x[t + off + j] * G[j]   (x zero-padded outside [0,T))
i.e. y = A_s @ x with the Toeplitz band matrix A_s[t, u] = G[u - t - off].

Kernel strategy v2 (SPMD over 8 cores): pure B-sharding.  Core c owns the
128 batch*channel columns [128c, 128c+128); every core computes all 2048
t_out rows for its columns.  All t_out-block indices are then
core-independent, so a single static instruction stream works with NO
per-core shifted data: the banded scales read x chunks directly (chunks
outside [0,16) are the zero-padding and are simply dropped).

All matmul operands are bfloat16 (full PE rate, half the DMA bytes of
fp32r; ~2.4e-3 rel err vs the 2e-2 budget).  Per (scale, t_out block I):
    psum[b, tau] += X_chunk[q].T @ Wsc[:, w0(s,I,q) : +512]
accumulated over the chunks q that intersect the band, where Wsc is a
per-scale Toeplitz sliding window (width max 128q-512I spread + 512),
identical on every core.  220 total matmuls/core (vs 240 in v1 — edge
chunks whose band falls outside [0,T) are skipped).

DMA: x (0.5MB) + W (3.0MB) in bf16, consumption-ordered pieces split
across the two HWDGE rings (sync=x+most outs, scalar=W+late outs); each
(s, I) output piece is copied psum->SBUF and DMA'd out as soon as its
chain stops.  The last chain is scale 1 / block 3 (5 matmuls) and its
copy+store is split across vector+scalar engines and both rings to
minimize the tail.
"""
import sys
import os

sys.path.insert(0, "/opt/trn_rl_repo")

import numpy as np

# ----------------------------------------------------------------- constants
WIDTHS = [1, 27, 76, 167, 336]
T = 2048
B = 1024  # 16 batch * 64 channels
N_CORES = 8
BPC = B // N_CORES  # 128 batch*channel columns per core
NQ = T // 128  # 16 t_in chunks
NI = T // 512  # 4 t_out blocks per core (all computed by every core)
NSC = len(WIDTHS)

LAST_EXEC_NS = None  # set when CWT_TRACE=1


def _filters():
    """pywt 'morl' integrated wavelet, resampled per scale (matches reference)."""
    precision = 10
    n = 2**precision
    lb, ub = -8.0, 8.0
    t = np.linspace(lb, ub, n)
    psi = np.exp(-(t**2) / 2.0) * np.cos(5.0 * t)
    step = t[1] - t[0]
    int_psi = np.cumsum(psi) * step
    filts = []
    for scale in WIDTHS:
        j = (np.arange(scale * (ub - lb) + 1) / (scale * step)).astype(np.int64)
        j = j[j < n]
        filts.append(int_psi[j].astype(np.float32))
    return filts


def _g_kernels():
    """Effective correlation kernels G_s (len L+1) and offsets off_s."""
    gs = []
    for s, k in zip(WIDTHS, _filters()):
        k64 = k.astype(np.float64)
        L = len(k64)
        G = (np.sqrt(s) * np.diff(np.concatenate([[0.0], k64, [0.0]]))).astype(
            np.float32
        )
        off = int(np.floor((L - 2) / 2.0)) - (L - 1)
        gs.append((G, off))
    return gs


def _plan():
    """Per-scale Toeplitz window geometry + per-block chunk ranges.

    w0(s, I, q) = C_s + off_s - (128q - 512I) is the first W column of the
    512-wide rhs slice for chunk q of t_out block I.
    """
    plans = []
    for G, off in _g_kernels():
        L1 = len(G)
        qr = []
        vs = []
        for I in range(NI):
            lo = max(0, (512 * I + off) // 128)
            hi = min(NQ - 1, (512 * I + 511 + off + L1 - 1) // 128)
            qr.append((lo, hi))
            vs += [128 * q - 512 * I for q in range(lo, hi + 1)]
        C = max(vs) - off
        W = max(vs) - min(vs) + 512
        plans.append({"off": off, "L1": L1, "qr": qr, "C": C, "W": W, "G": G})
    return plans


def _toeplitz(G, C, W):
    p = np.arange(128)[:, None]
    w = np.arange(W)[None, :]
    idx = p - w + C
    valid = (idx >= 0) & (idx < len(G))
    return np.where(valid, G[np.clip(idx, 0, len(G) - 1)], np.float32(0.0)).astype(
        np.float32
    )


# chain processing groups: (scale, t_out block).  Scale 1's short chains
# bracket the schedule: two start it (smallest W piece -> earliest first
# matmul) and two end it (shortest chains -> smallest output tail).  The
# dense scales run mid-schedule so their 4-chains-end-together output
# bursts drain while later groups compute.
GROUPS = [
    [(0, 0), (0, 1)],
    [(1, 0), (1, 1), (1, 2), (1, 3)],
    [(3, 0), (3, 1), (3, 2), (3, 3)],
    [(4, 0), (4, 1), (4, 2), (4, 3)],
    [(2, 0), (2, 1), (2, 2), (2, 3)],
    [(0, 2), (0, 3)],
]


def _chain_windows(p, I):
    """Per-chunk banded column windows for one (scale, block) chain.

    Returns [(q, c0, c1, start, stop)]: matmul psum cols [c0, c1) with
    uniform first/last-toucher flags, skipping the all-zero columns of the
    Toeplitz band slice (big win for the narrow-band scales).
    """
    C, L1, off = p["C"], p["L1"], p["off"]
    nz_lo, nz_hi = C - L1 + 1, C + 128  # nonzero W cols [lo, hi)
    lo, hi = p["qr"][I]
    spans = {}
    first = {}
    last = {}
    for q in range(lo, hi + 1):
        w0 = C + off - (128 * q - 512 * I)
        a = max(0, nz_lo - w0)
        b = min(512, nz_hi - w0)
        if b <= a:
            continue
        jlo, jhi = a // 128, (b + 127) // 128
        spans[q] = (jlo, jhi)
        for j in range(jlo, jhi):
            first.setdefault(j, q)
            last[j] = q
    # start_tensor_calc resets the ENTIRE psum bank on hardware, so each
    # chain gets exactly one start: its first matmul, widened to the full
    # column union of the chain.  stop is sim-only bookkeeping; it rides
    # the final matmul.
    qs = list(spans)
    q0 = qs[0]
    jmin = min(j for j, _ in spans.values())
    jmax = max(j for _, j in spans.values())
    out = [(q0, jmin * 128, jmax * 128, True, len(qs) == 1)]
    for i, q in enumerate(qs[1:]):
        jlo, jhi = spans[q]
        out.append((q, jlo * 128, jhi * 128, False, i == len(qs) - 2))
    return out


def _schedule(plans):
    """Emission-ordered list of (s, I, q, w0, c0, c1, start, stop, chain_done)."""
    sched = []
    for chains in GROUPS:
        wins = {(s, I): _chain_windows(plans[s], I) for s, I in chains}
        left = {k: len(v) for k, v in wins.items()}
        qmin = min(q for v in wins.values() for q, *_ in v)
        qmax = max(q for v in wins.values() for q, *_ in v)
        for q in range(qmin, qmax + 1):
            for s, I in chains:
                for qq, c0, c1, st, sp in wins[(s, I)]:
                    if qq != q:
                        continue
                    w0 = plans[s]["C"] + plans[s]["off"] - (128 * q - 512 * I)
                    left[(s, I)] -= 1
                    sched.append(
                        (s, I, q, w0, c0, c1, st, sp, left[(s, I)] == 0)
                    )
    return sched


_CONST_CACHE = None


def _consts():
    global _CONST_CACHE
    if _CONST_CACHE is None:
        import ml_dtypes

        plans = _plan()
        wbuf = np.concatenate(
            [_toeplitz(p["G"], p["C"], p["W"]) for p in plans], axis=1
        ).astype(ml_dtypes.bfloat16)
        _CONST_CACHE = (plans, np.ascontiguousarray(wbuf))
    return _CONST_CACHE


# ----------------------------------------------------------------- program
_NC_CACHE = None


def _w_pieces(plans, sched):
    """Split the concatenated W buffer into DMA pieces, ordered by first use.

    Scale 0's first piece is cut to exactly cover its first two schedule
    steps so the PE can start ~1us sooner; other scales use 2 halves
    (few pieces -> few DMA sems -> short semaphore-teardown epilogue).
    """
    bases = []
    b = 0
    for p in plans:
        bases.append(b)
        b += p["W"]
    pieces = []  # (first_use_idx, lo, hi) in concat cols
    for s, p in enumerate(plans):
        # only the columns the windowed schedule actually reads
        rd = [(w0 + c0, w0 + c1) for ss, I, q, w0, c0, c1, st, sp, cd in sched if ss == s]
        rlo = min(a for a, b in rd)
        rhi = max(b for a, b in rd)
        npieces = max(1, round((rhi - rlo) / 1024))
        step = (((rhi - rlo) // npieces) + 127) & ~127
        cuts = list(range(rlo, rhi, step)) + [rhi]
        for lo, hi in zip(cuts[:-1], cuts[1:]):
            first = None
            for i, (ss, I, q, w0, c0, c1, st, sp, cd) in enumerate(sched):
                if ss == s and w0 + c0 < hi and w0 + c1 > lo:
                    first = i
                    break
            pieces.append((first if first is not None else len(sched), bases[s] + lo, bases[s] + hi))
    pieces.sort()
    return bases, [(lo, hi) for _, lo, hi in pieces]


def _build_program():
    import concourse.bass as bass
    import concourse.bacc as bacc
    import concourse.mybir as mybir
    import concourse.tile as tile

    plans, _ = _consts()
    sched = _schedule(plans)
    wtot = sum(p["W"] for p in plans)
    bases, wpieces = _w_pieces(plans, sched)

    nc = bacc.Bacc(None, target_bir_lowering=False, debug=False)

    x_d = nc.declare_dram_parameter("x", [128, NQ * BPC], mybir.dt.bfloat16, isOutput=False)
    w_d = nc.declare_dram_parameter("w", [128, wtot], mybir.dt.bfloat16, isOutput=False)
    # outputs leave as bf16 (host upcasts): halves the store bytes; adds
    # ~0.1% rms quantization vs the 2e-2 budget
    out_d = nc.declare_dram_parameter(
        "out", [NSC, 128, T], mybir.dt.bfloat16, isOutput=True
    )

    # ring for each whole-scale output store (sync carries x early, scalar
    # carries W early; both are free by the time these flow mid-schedule)
    OUT_ENG = {1: "sync", 3: "sync", 4: "scalar", 2: "scalar"}
    LAST_CHAIN = (0, 3)

    with tile.TileContext(nc) as tc:
        with (
            tc.tile_pool(name="xp", bufs=1) as xp,
            tc.tile_pool(name="wp", bufs=1) as wp,
            tc.tile_pool(name="op", bufs=1) as op,
            tc.tile_pool(name="pp", bufs=1, space=bass.MemorySpace.PSUM) as pp,
        ):
            # x chunks on the sync (SP) HWDGE ring in consumption order
            xsb = xp.tile([128, NQ * BPC], mybir.dt.bfloat16, tag="xsb", name="xsb")
            for g0, g1 in ((0, 3), (3, 7), (7, 11), (11, NQ)):
                nc.sync.dma_start(
                    xsb[:, g0 * BPC : g1 * BPC],
                    x_d[:, g0 * BPC : g1 * BPC],
                )

            # W pieces on the scalar (ACT) ring, first-use order
            wsb = wp.tile([128, wtot], mybir.dt.bfloat16, tag="wsb", name="wsb")
            for lo, hi in wpieces:
                nc.scalar.dma_start(wsb[:, lo:hi], w_d[:, lo:hi])

            stgs = [
                op.tile([128, T], mybir.dt.bfloat16, tag=f"stg{s}", name=f"stg{s}")
                for s in range(NSC)
            ]

            # psum tags: groups alternate the two 4-bank halves
            psums = {}
            for gi, chains in enumerate(GROUPS):
                for ci, (s, I) in enumerate(chains):
                    psums[(s, I)] = pp.tile(
                        [128, 512],
                        mybir.dt.float32,
                        tag=f"ps{(gi % 2) * 4 + ci}",
                        name=f"ps_{s}_{I}",
                    )

            done = {s: 0 for s in range(NSC)}
            for s, I, q, w0, c0, c1, start, stop, chain_done in sched:
                nc.tensor.matmul(
                    psums[(s, I)][:, c0:c1],
                    xsb[:, q * BPC : (q + 1) * BPC],
                    wsb[:, bases[s] + w0 + c0 : bases[s] + w0 + c1],
                    start=start,
                    stop=stop,
                )
                if not chain_done:
                    continue
                stg = stgs[s]
                t0c = 512 * I
                done[s] += 1
                nc.vector.tensor_copy(stg[:, t0c : t0c + 512], psums[(s, I)][:])
                if s == 0:
                    # scale 1 brackets the schedule; store per-block pieces
                    if (s, I) == LAST_CHAIN:
                        # split the final store across both rings to shorten
                        # the tail
                        nc.sync.dma_start(
                            out_d[s][:, t0c : t0c + 256], stg[:, t0c : t0c + 256]
                        )
                        nc.scalar.dma_start(
                            out_d[s][:, t0c + 256 : t0c + 512],
                            stg[:, t0c + 256 : t0c + 512],
                        )
                    else:
                        nc.sync.dma_start(
                            out_d[s][:, t0c : t0c + 512], stg[:, t0c : t0c + 512]
                        )
                else:
                    if done[s] == NI:
                        # whole-scale store once the last block is staged
                        eng = nc.sync if OUT_ENG[s] == "sync" else nc.scalar
                        eng.dma_start(out_d[s], stg[:])

    nc.compile()
    return nc


def _program():
    global _NC_CACHE
    if _NC_CACHE is None:
        _NC_CACHE = _build_program()
    return _NC_CACHE


# ----------------------------------------------------------------- entry
def kernel(x: np.ndarray) -> np.ndarray:
    """x: [16, 2048, 64] float32 -> [16, 2048, 64, 5] float32"""
    global LAST_EXEC_NS
    import ml_dtypes
    from concourse.bass_utils import run_bass_kernel_spmd

    n, t, c = x.shape
    assert (t, n * c) == (T, B), (x.shape,)

    X = x.transpose(1, 0, 2).reshape(T, B).astype(np.float32)
    _, wbuf = _consts()
    in_maps = []
    for core in range(N_CORES):
        xc = X[:, core * BPC : (core + 1) * BPC]  # [2048, 128]
        xc = (
            xc.reshape(NQ, 128, BPC)
            .transpose(1, 0, 2)
            .reshape(128, NQ * BPC)
            .astype(ml_dtypes.bfloat16)
        )
        in_maps.append({"x": np.ascontiguousarray(xc), "w": wbuf})

    nc = _program()
    trace = bool(int(os.environ.get("CWT_TRACE", "0")))
    res = run_bass_kernel_spmd(nc, in_maps, list(range(N_CORES)), trace=trace)
    if trace:
        LAST_EXEC_NS = res.exec_time_ns
        globals()["LAST_RESULTS"] = res

    # per-core out: [5, 128, 2048] bf16 (b-local, t) -> Y [5, T, B] fp32
    Y = np.empty((NSC, T, B), np.float32)
    for core in range(N_CORES):
        o = np.asarray(res.results[core]["out"]).astype(np.float32)
        Y[:, :, core * BPC : (core + 1) * BPC] = o.transpose(0, 2, 1)
    return np.ascontiguousarray(
        Y.reshape(NSC, T, n, c).transpose(2, 1, 3, 0).astype(np.float32)
    )


# revision 15
# speedup vs baseline: 1.3968x; 1.0535x over previous
"""CWT (continuous wavelet transform, pywt 'morl', 5 scales) as a Bass/Tile
kernel for 8 Trainium2 NeuronCores.

Math: for each scale s with integrated-wavelet filter k (length L), the
reference computes  trim(diff(full_corr(x, k))) * (-sqrt(s)) along T.  That
whole pipeline is a single correlation with the fixed kernel
    G[j] = sqrt(s) * (k[j] - k[j-1]),  j = 0..L  (k[-1] = k[L] = 0)
applied with offset  off = floor((L-2)/2) - (L-1):
    y[t] = sum_j x[t + off + j] * G[j]   (x zero-padded outside [0,T))
i.e. y = A_s @ x with the Toeplitz band matrix A_s[t, u] = G[u - t - off].

Kernel strategy v2 (SPMD over 8 cores): pure B-sharding.  Core c owns the
128 batch*channel columns [128c, 128c+128); every core computes all 2048
t_out rows for its columns.  All t_out-block indices are then
core-independent, so a single static instruction stream works with NO
per-core shifted data: the banded scales read x chunks directly (chunks
outside [0,16) are the zero-padding and are simply dropped).

All matmul operands are bfloat16 (full PE rate, half the DMA bytes of
fp32r; ~2.4e-3 rel err vs the 2e-2 budget).  Per (scale, t_out block I):
    psum[b, tau] += X_chunk[q].T @ Wsc[:, w0(s,I,q) : +512]
accumulated over the chunks q that intersect the band, where Wsc is a
per-scale Toeplitz sliding window (width max 128q-512I spread + 512),
identical on every core.  220 total matmuls/core (vs 240 in v1 — edge
chunks whose band falls outside [0,T) are skipped).

DMA: x (0.5MB) + W (3.0MB) in bf16, consumption-ordered pieces split
across the two HWDGE rings (sync=x+most outs, scalar=W+late outs); each
(s, I) output piece is copied psum->SBUF and DMA'd out as soon as its
chain stops.  The last chain is scale 1 / block 3 (5 matmuls) and its
copy+store is split across vector+scalar engines and both rings to
minimize the tail.
"""
import sys
import os

sys.path.insert(0, "/opt/trn_rl_repo")

import numpy as np

# ----------------------------------------------------------------- constants
WIDTHS = [1, 27, 76, 167, 336]
T = 2048
B = 1024  # 16 batch * 64 channels
N_CORES = 8
BPC = B // N_CORES  # 128 batch*channel columns per core
NQ = T // 128  # 16 t_in chunks
NI = T // 512  # 4 t_out blocks per core (all computed by every core)
NSC = len(WIDTHS)

LAST_EXEC_NS = None  # set when CWT_TRACE=1


def _filters():
    """pywt 'morl' integrated wavelet, resampled per scale (matches reference)."""
    precision = 10
    n = 2**precision
    lb, ub = -8.0, 8.0
    t = np.linspace(lb, ub, n)
    psi = np.exp(-(t**2) / 2.0) * np.cos(5.0 * t)
    step = t[1] - t[0]
    int_psi = np.cumsum(psi) * step
    filts = []
    for scale in WIDTHS:
        j = (np.arange(scale * (ub - lb) + 1) / (scale * step)).astype(np.int64)
        j = j[j < n]
        filts.append(int_psi[j].astype(np.float32))
    return filts


def _g_kernels():
    """Effective correlation kernels G_s (len L+1) and offsets off_s."""
    gs = []
    for s, k in zip(WIDTHS, _filters()):
        k64 = k.astype(np.float64)
        L = len(k64)
        G = (np.sqrt(s) * np.diff(np.concatenate([[0.0], k64, [0.0]]))).astype(
            np.float32
        )
        off = int(np.floor((L - 2) / 2.0)) - (L - 1)
        gs.append((G, off))
    return gs


def _plan():
    """Per-scale Toeplitz window geometry + per-block chunk ranges.

    w0(s, I, q) = C_s + off_s - (128q - 512I) is the first W column of the
    512-wide rhs slice for chunk q of t_out block I.
    """
    plans = []
    for G, off in _g_kernels():
        L1 = len(G)
        qr = []
        vs = []
        for I in range(NI):
            lo = max(0, (512 * I + off) // 128)
            hi = min(NQ - 1, (512 * I + 511 + off + L1 - 1) // 128)
            qr.append((lo, hi))
            vs += [128 * q - 512 * I for q in range(lo, hi + 1)]
        C = max(vs) - off
        W = max(vs) - min(vs) + 512
        plans.append({"off": off, "L1": L1, "qr": qr, "C": C, "W": W, "G": G})
    return plans


def _toeplitz(G, C, W):
    p = np.arange(128)[:, None]
    w = np.arange(W)[None, :]
    idx = p - w + C
    valid = (idx >= 0) & (idx < len(G))
    return np.where(valid, G[np.clip(idx, 0, len(G) - 1)], np.float32(0.0)).astype(
        np.float32
    )


# chain processing groups: (scale, t_out block).  Scale 1's short chains
# bracket the schedule: two start it (smallest W piece -> earliest first
# matmul) and two end it (shortest chains -> smallest output tail).  The
# dense scales run mid-schedule so their 4-chains-end-together output
# bursts drain while later groups compute.
GROUPS = [
    [(0, 0), (0, 1)],
    [(1, 0), (1, 1), (1, 2), (1, 3)],
    [(3, 0), (3, 1), (3, 2), (3, 3)],
    [(4, 0), (4, 1), (4, 2), (4, 3)],
    [(2, 0), (2, 1), (2, 2), (2, 3)],
    [(0, 2), (0, 3)],
]


def _chain_windows(p, I):
    """Per-chunk banded column windows for one (scale, block) chain.

    Returns [(q, c0, c1, start, stop)]: matmul psum cols [c0, c1) with
    uniform first/last-toucher flags, skipping the all-zero columns of the
    Toeplitz band slice (big win for the narrow-band scales).
    """
    C, L1, off = p["C"], p["L1"], p["off"]
    nz_lo, nz_hi = C - L1 + 1, C + 128  # nonzero W cols [lo, hi)
    lo, hi = p["qr"][I]
    spans = {}
    first = {}
    last = {}
    for q in range(lo, hi + 1):
        w0 = C + off - (128 * q - 512 * I)
        a = max(0, nz_lo - w0)
        b = min(512, nz_hi - w0)
        if b <= a:
            continue
        jlo, jhi = a // 128, (b + 127) // 128
        spans[q] = (jlo, jhi)
        for j in range(jlo, jhi):
            first.setdefault(j, q)
            last[j] = q
    # start_tensor_calc resets the ENTIRE psum bank on hardware, so each
    # chain gets exactly one start.  Accumulation is commutative: lead with
    # the chunk whose band window is widest (interior chunks cover the full
    # 512, so widening the start to the chain's column union is ~free) and
    # append the remaining chunks in q order.  stop rides the final matmul.
    qs = list(spans)
    qstar = max(qs, key=lambda q: spans[q][1] - spans[q][0])
    jmin = min(j for j, _ in spans.values())
    jmax = max(j for _, j in spans.values())
    rest = [q for q in qs if q != qstar]
    out = [(qstar, jmin * 128, jmax * 128, True, not rest)]
    for i, q in enumerate(rest):
        jlo, jhi = spans[q]
        out.append((q, jlo * 128, jhi * 128, False, i == len(rest) - 1))
    return out


def _schedule(plans):
    """Emission-ordered list of (s, I, q, w0, c0, c1, start, stop, chain_done).

    Chains in a group are interleaved round-robin by window index, which
    preserves each chain's start-first order while pacing x-chunk arrival.
    """
    sched = []
    for chains in GROUPS:
        wins = {(s, I): _chain_windows(plans[s], I) for s, I in chains}
        maxlen = max(len(v) for v in wins.values())
        for step in range(maxlen):
            for s, I in chains:
                v = wins[(s, I)]
                if step >= len(v):
                    continue
                q, c0, c1, st, sp = v[step]
                w0 = plans[s]["C"] + plans[s]["off"] - (128 * q - 512 * I)
                sched.append((s, I, q, w0, c0, c1, st, sp, step == len(v) - 1))
    return sched


_CONST_CACHE = None


def _consts():
    global _CONST_CACHE
    if _CONST_CACHE is None:
        import ml_dtypes

        plans = _plan()
        wbuf = np.concatenate(
            [_toeplitz(p["G"], p["C"], p["W"]) for p in plans], axis=1
        ).astype(ml_dtypes.bfloat16)
        _CONST_CACHE = (plans, np.ascontiguousarray(wbuf))
    return _CONST_CACHE


# ----------------------------------------------------------------- program
_NC_CACHE = None


def _w_pieces(plans, sched):
    """Split the concatenated W buffer into DMA pieces, ordered by first use.

    Scale 0's first piece is cut to exactly cover its first two schedule
    steps so the PE can start ~1us sooner; other scales use 2 halves
    (few pieces -> few DMA sems -> short semaphore-teardown epilogue).
    """
    bases = []
    b = 0
    for p in plans:
        bases.append(b)
        b += p["W"]
    pieces = []  # (first_use_idx, lo, hi) in concat cols
    for s, p in enumerate(plans):
        # only the columns the windowed schedule actually reads
        rd = [(w0 + c0, w0 + c1) for ss, I, q, w0, c0, c1, st, sp, cd in sched if ss == s]
        rlo = min(a for a, b in rd)
        rhi = max(b for a, b in rd)
        npieces = max(1, round((rhi - rlo) / 1024))
        step = (((rhi - rlo) // npieces) + 127) & ~127
        cuts = list(range(rlo, rhi, step)) + [rhi]
        for lo, hi in zip(cuts[:-1], cuts[1:]):
            first = None
            for i, (ss, I, q, w0, c0, c1, st, sp, cd) in enumerate(sched):
                if ss == s and w0 + c0 < hi and w0 + c1 > lo:
                    first = i
                    break
            pieces.append((first if first is not None else len(sched), bases[s] + lo, bases[s] + hi))
    pieces.sort()
    return bases, [(lo, hi) for _, lo, hi in pieces]


def _build_program():
    import concourse.bass as bass
    import concourse.bacc as bacc
    import concourse.mybir as mybir
    import concourse.tile as tile

    plans, _ = _consts()
    sched = _schedule(plans)
    wtot = sum(p["W"] for p in plans)
    bases, wpieces = _w_pieces(plans, sched)

    nc = bacc.Bacc(None, target_bir_lowering=False, debug=False)

    x_d = nc.declare_dram_parameter("x", [128, NQ * BPC], mybir.dt.bfloat16, isOutput=False)
    w_d = nc.declare_dram_parameter("w", [128, wtot], mybir.dt.bfloat16, isOutput=False)
    # outputs leave as bf16 (host upcasts): halves the store bytes; adds
    # ~0.1% rms quantization vs the 2e-2 budget
    out_d = nc.declare_dram_parameter(
        "out", [NSC, 128, T], mybir.dt.bfloat16, isOutput=True
    )

    # ring for each whole-scale output store (sync carries x early, scalar
    # carries W early; both are free by the time these flow mid-schedule)
    OUT_ENG = {1: "sync", 3: "sync", 4: "scalar", 2: "scalar"}
    LAST_CHAIN = (0, 3)

    with tile.TileContext(nc) as tc:
        with (
            tc.tile_pool(name="xp", bufs=1) as xp,
            tc.tile_pool(name="wp", bufs=1) as wp,
            tc.tile_pool(name="op", bufs=1) as op,
            tc.tile_pool(name="pp", bufs=1, space=bass.MemorySpace.PSUM) as pp,
        ):
            # x chunks on the sync (SP) HWDGE ring in consumption order
            xsb = xp.tile([128, NQ * BPC], mybir.dt.bfloat16, tag="xsb", name="xsb")
            for g0, g1 in ((0, 3), (3, 7), (7, 11), (11, NQ)):
                nc.sync.dma_start(
                    xsb[:, g0 * BPC : g1 * BPC],
                    x_d[:, g0 * BPC : g1 * BPC],
                )

            # W pieces on the scalar (ACT) ring, first-use order
            wsb = wp.tile([128, wtot], mybir.dt.bfloat16, tag="wsb", name="wsb")
            for lo, hi in wpieces:
                nc.scalar.dma_start(wsb[:, lo:hi], w_d[:, lo:hi])

            stgs = [
                op.tile([128, T], mybir.dt.bfloat16, tag=f"stg{s}", name=f"stg{s}")
                for s in range(NSC)
            ]

            # psum tags: groups alternate the two 4-bank halves
            psums = {}
            for gi, chains in enumerate(GROUPS):
                for ci, (s, I) in enumerate(chains):
                    psums[(s, I)] = pp.tile(
                        [128, 512],
                        mybir.dt.float32,
                        tag=f"ps{(gi % 2) * 4 + ci}",
                        name=f"ps_{s}_{I}",
                    )

            done = {s: 0 for s in range(NSC)}
            for s, I, q, w0, c0, c1, start, stop, chain_done in sched:
                nc.tensor.matmul(
                    psums[(s, I)][:, c0:c1],
                    xsb[:, q * BPC : (q + 1) * BPC],
                    wsb[:, bases[s] + w0 + c0 : bases[s] + w0 + c1],
                    start=start,
                    stop=stop,
                )
                if not chain_done:
                    continue
                stg = stgs[s]
                t0c = 512 * I
                done[s] += 1
                nc.vector.tensor_copy(stg[:, t0c : t0c + 512], psums[(s, I)][:])
                if s == 0:
                    # scale 1 brackets the schedule; store per-block pieces
                    if (s, I) == LAST_CHAIN:
                        # split the final store across both rings to shorten
                        # the tail
                        nc.sync.dma_start(
                            out_d[s][:, t0c : t0c + 256], stg[:, t0c : t0c + 256]
                        )
                        nc.scalar.dma_start(
                            out_d[s][:, t0c + 256 : t0c + 512],
                            stg[:, t0c + 256 : t0c + 512],
                        )
                    else:
                        nc.sync.dma_start(
                            out_d[s][:, t0c : t0c + 512], stg[:, t0c : t0c + 512]
                        )
                else:
                    if done[s] == NI:
                        # whole-scale store once the last block is staged
                        eng = nc.sync if OUT_ENG[s] == "sync" else nc.scalar
                        eng.dma_start(out_d[s], stg[:])

    nc.compile()
    return nc


def _program():
    global _NC_CACHE
    if _NC_CACHE is None:
        _NC_CACHE = _build_program()
    return _NC_CACHE


# ----------------------------------------------------------------- entry
def kernel(x: np.ndarray) -> np.ndarray:
    """x: [16, 2048, 64] float32 -> [16, 2048, 64, 5] float32"""
    global LAST_EXEC_NS
    import ml_dtypes
    from concourse.bass_utils import run_bass_kernel_spmd

    n, t, c = x.shape
    assert (t, n * c) == (T, B), (x.shape,)

    X = x.transpose(1, 0, 2).reshape(T, B).astype(np.float32)
    _, wbuf = _consts()
    in_maps = []
    for core in range(N_CORES):
        xc = X[:, core * BPC : (core + 1) * BPC]  # [2048, 128]
        xc = (
            xc.reshape(NQ, 128, BPC)
            .transpose(1, 0, 2)
            .reshape(128, NQ * BPC)
            .astype(ml_dtypes.bfloat16)
        )
        in_maps.append({"x": np.ascontiguousarray(xc), "w": wbuf})

    nc = _program()
    trace = bool(int(os.environ.get("CWT_TRACE", "0")))
    res = run_bass_kernel_spmd(nc, in_maps, list(range(N_CORES)), trace=trace)
    if trace:
        LAST_EXEC_NS = res.exec_time_ns
        globals()["LAST_RESULTS"] = res

    # per-core out: [5, 128, 2048] bf16 (b-local, t) -> Y [5, T, B] fp32
    Y = np.empty((NSC, T, B), np.float32)
    for core in range(N_CORES):
        o = np.asarray(res.results[core]["out"]).astype(np.float32)
        Y[:, :, core * BPC : (core + 1) * BPC] = o.transpose(0, 2, 1)
    return np.ascontiguousarray(
        Y.reshape(NSC, T, n, c).transpose(2, 1, 3, 0).astype(np.float32)
    )


# revision 19
# speedup vs baseline: 1.4494x; 1.0376x over previous
"""CWT (continuous wavelet transform, pywt 'morl', 5 scales) as a Bass/Tile
kernel for 8 Trainium2 NeuronCores.

Math: for each scale s with integrated-wavelet filter k (length L), the
reference computes  trim(diff(full_corr(x, k))) * (-sqrt(s)) along T.  That
whole pipeline is a single correlation with the fixed kernel
    G[j] = sqrt(s) * (k[j] - k[j-1]),  j = 0..L  (k[-1] = k[L] = 0)
applied with offset  off = floor((L-2)/2) - (L-1):
    y[t] = sum_j x[t + off + j] * G[j]   (x zero-padded outside [0,T))
i.e. y = A_s @ x with the Toeplitz band matrix A_s[t, u] = G[u - t - off].

Kernel strategy v2 (SPMD over 8 cores): pure B-sharding.  Core c owns the
128 batch*channel columns [128c, 128c+128); every core computes all 2048
t_out rows for its columns.  All t_out-block indices are then
core-independent, so a single static instruction stream works with NO
per-core shifted data: the banded scales read x chunks directly (chunks
outside [0,16) are the zero-padding and are simply dropped).

All matmul operands are bfloat16 (full PE rate, half the DMA bytes of
fp32r; ~2.4e-3 rel err vs the 2e-2 budget).  Per (scale, t_out block I):
    psum[b, tau] += X_chunk[q].T @ Wsc[:, w0(s,I,q) : +512]
accumulated over the chunks q that intersect the band, where Wsc is a
per-scale Toeplitz sliding window (width max 128q-512I spread + 512),
identical on every core.  220 total matmuls/core (vs 240 in v1 — edge
chunks whose band falls outside [0,T) are skipped).

DMA: x (0.5MB) + W (3.0MB) in bf16, consumption-ordered pieces split
across the two HWDGE rings (sync=x+most outs, scalar=W+late outs); each
(s, I) output piece is copied psum->SBUF and DMA'd out as soon as its
chain stops.  The last chain is scale 1 / block 3 (5 matmuls) and its
copy+store is split across vector+scalar engines and both rings to
minimize the tail.
"""
import sys
import os

sys.path.insert(0, "/opt/trn_rl_repo")

import numpy as np

# ----------------------------------------------------------------- constants
WIDTHS = [1, 27, 76, 167, 336]
T = 2048
B = 1024  # 16 batch * 64 channels
N_CORES = 8
BPC = B // N_CORES  # 128 batch*channel columns per core
NQ = T // 128  # 16 t_in chunks
NI = T // 512  # 4 t_out blocks per core (all computed by every core)
NSC = len(WIDTHS)

LAST_EXEC_NS = None  # set when CWT_TRACE=1


def _filters():
    """pywt 'morl' integrated wavelet, resampled per scale (matches reference)."""
    precision = 10
    n = 2**precision
    lb, ub = -8.0, 8.0
    t = np.linspace(lb, ub, n)
    psi = np.exp(-(t**2) / 2.0) * np.cos(5.0 * t)
    step = t[1] - t[0]
    int_psi = np.cumsum(psi) * step
    filts = []
    for scale in WIDTHS:
        j = (np.arange(scale * (ub - lb) + 1) / (scale * step)).astype(np.int64)
        j = j[j < n]
        filts.append(int_psi[j].astype(np.float32))
    return filts


def _g_kernels():
    """Effective correlation kernels G_s (len L+1) and offsets off_s."""
    gs = []
    for s, k in zip(WIDTHS, _filters()):
        k64 = k.astype(np.float64)
        L = len(k64)
        G = (np.sqrt(s) * np.diff(np.concatenate([[0.0], k64, [0.0]]))).astype(
            np.float32
        )
        off = int(np.floor((L - 2) / 2.0)) - (L - 1)
        gs.append((G, off))
    return gs


def _plan():
    """Per-scale Toeplitz window geometry + per-block chunk ranges.

    w0(s, I, q) = C_s + off_s - (128q - 512I) is the first W column of the
    512-wide rhs slice for chunk q of t_out block I.
    """
    plans = []
    for G, off in _g_kernels():
        L1 = len(G)
        qr = []
        vs = []
        for I in range(NI):
            lo = max(0, (512 * I + off) // 128)
            hi = min(NQ - 1, (512 * I + 511 + off + L1 - 1) // 128)
            qr.append((lo, hi))
            vs += [128 * q - 512 * I for q in range(lo, hi + 1)]
        C = max(vs) - off
        W = max(vs) - min(vs) + 512
        plans.append({"off": off, "L1": L1, "qr": qr, "C": C, "W": W, "G": G})
    return plans


def _toeplitz(G, C, W):
    p = np.arange(128)[:, None]
    w = np.arange(W)[None, :]
    idx = p - w + C
    valid = (idx >= 0) & (idx < len(G))
    return np.where(valid, G[np.clip(idx, 0, len(G) - 1)], np.float32(0.0)).astype(
        np.float32
    )


# chain processing groups: (scale, t_out block).  Scale 1's short chains
# bracket the schedule: two start it (smallest W piece -> earliest first
# matmul) and two end it (shortest chains -> smallest output tail).  The
# dense scales run mid-schedule so their 4-chains-end-together output
# bursts drain while later groups compute.
GROUPS = [
    [(0, 0), (0, 1)],
    [(1, 0), (1, 1), (1, 2), (1, 3)],
    [(3, 0), (3, 1), (3, 2), (3, 3)],
    [(4, 0), (4, 1), (4, 2), (4, 3)],
    [(2, 0), (2, 1), (2, 2), (2, 3)],
    [(0, 2), (0, 3)],
]


def _chain_windows(p, I):
    """Per-chunk banded column windows for one (scale, block) chain.

    Returns [(q, c0, c1, start, stop)]: matmul psum cols [c0, c1) with
    uniform first/last-toucher flags, skipping the all-zero columns of the
    Toeplitz band slice (big win for the narrow-band scales).
    """
    C, L1, off = p["C"], p["L1"], p["off"]
    nz_lo, nz_hi = C - L1 + 1, C + 128  # nonzero W cols [lo, hi)
    lo, hi = p["qr"][I]
    spans = {}
    first = {}
    last = {}
    for q in range(lo, hi + 1):
        w0 = C + off - (128 * q - 512 * I)
        a = max(0, nz_lo - w0)
        b = min(512, nz_hi - w0)
        if b <= a:
            continue
        jlo, jhi = a // 128, (b + 127) // 128
        spans[q] = (jlo, jhi)
        for j in range(jlo, jhi):
            first.setdefault(j, q)
            last[j] = q
    # start_tensor_calc resets the ENTIRE psum bank on hardware, so each
    # chain gets exactly one start.  Accumulation is commutative: lead with
    # the chunk whose band window is widest (interior chunks cover the full
    # 512, so widening the start to the chain's column union is ~free) and
    # append the remaining chunks in q order.  stop rides the final matmul.
    qs = list(spans)
    qstar = max(qs, key=lambda q: spans[q][1] - spans[q][0])
    jmin = min(j for j, _ in spans.values())
    jmax = max(j for _, j in spans.values())
    rest = [q for q in qs if q != qstar]
    out = [(qstar, jmin * 128, jmax * 128, True, not rest)]
    for i, q in enumerate(rest):
        jlo, jhi = spans[q]
        out.append((q, jlo * 128, jhi * 128, False, i == len(rest) - 1))
    return out


def _schedule(plans):
    """Emission-ordered list of (s, I, q, w0, c0, c1, start, stop, chain_done).

    Chains in a group are interleaved round-robin by window index, which
    preserves each chain's start-first order while pacing x-chunk arrival.
    """
    sched = []
    for chains in GROUPS:
        wins = {(s, I): _chain_windows(plans[s], I) for s, I in chains}
        maxlen = max(len(v) for v in wins.values())
        for step in range(maxlen):
            for s, I in chains:
                v = wins[(s, I)]
                if step >= len(v):
                    continue
                q, c0, c1, st, sp = v[step]
                w0 = plans[s]["C"] + plans[s]["off"] - (128 * q - 512 * I)
                sched.append((s, I, q, w0, c0, c1, st, sp, step == len(v) - 1))
    return sched


_CONST_CACHE = None


def _consts():
    global _CONST_CACHE
    if _CONST_CACHE is None:
        import ml_dtypes

        plans = _plan()
        wbuf = np.concatenate(
            [_toeplitz(p["G"], p["C"], p["W"]) for p in plans], axis=1
        ).astype(ml_dtypes.bfloat16)
        _CONST_CACHE = (plans, np.ascontiguousarray(wbuf))
    return _CONST_CACHE


# ----------------------------------------------------------------- program
_NC_CACHE = None


def _w_pieces(plans, sched):
    """Split the concatenated W buffer into DMA pieces, ordered by first use.

    Scale 0's first piece is cut to exactly cover its first two schedule
    steps so the PE can start ~1us sooner; other scales use 2 halves
    (few pieces -> few DMA sems -> short semaphore-teardown epilogue).
    """
    bases = []
    b = 0
    for p in plans:
        bases.append(b)
        b += p["W"]
    pieces = []  # (first_use_idx, lo, hi) in concat cols
    for s, p in enumerate(plans):
        # only the columns the windowed schedule actually reads
        rd = [(w0 + c0, w0 + c1) for ss, I, q, w0, c0, c1, st, sp, cd in sched if ss == s]
        rlo = min(a for a, b in rd)
        rhi = max(b for a, b in rd)
        npieces = max(1, round((rhi - rlo) / 1024))
        step = (((rhi - rlo) // npieces) + 127) & ~127
        cuts = list(range(rlo, rhi, step)) + [rhi]
        for lo, hi in zip(cuts[:-1], cuts[1:]):
            first = None
            for i, (ss, I, q, w0, c0, c1, st, sp, cd) in enumerate(sched):
                if ss == s and w0 + c0 < hi and w0 + c1 > lo:
                    first = i
                    break
            pieces.append((first if first is not None else len(sched), bases[s] + lo, bases[s] + hi))
    pieces.sort()
    return bases, [(lo, hi) for _, lo, hi in pieces]


def _build_program():
    import concourse.bass as bass
    import concourse.bacc as bacc
    import concourse.mybir as mybir
    import concourse.tile as tile

    plans, _ = _consts()
    sched = _schedule(plans)
    wtot = sum(p["W"] for p in plans)
    bases, wpieces = _w_pieces(plans, sched)

    nc = bacc.Bacc(None, target_bir_lowering=False, debug=False)

    x_d = nc.declare_dram_parameter("x", [128, NQ * BPC], mybir.dt.bfloat16, isOutput=False)
    w_d = nc.declare_dram_parameter("w", [128, wtot], mybir.dt.bfloat16, isOutput=False)
    # outputs leave as bf16 (host upcasts): halves the store bytes; adds
    # ~0.1% rms quantization vs the 2e-2 budget
    out_d = nc.declare_dram_parameter(
        "out", [NSC, 128, T], mybir.dt.bfloat16, isOutput=True
    )

    # ring for each whole-scale output store (sync carries x early, scalar
    # carries W early; both are free by the time these flow mid-schedule)
    OUT_ENG = {1: "sync", 3: "sync", 4: "scalar", 2: "scalar"}
    # (0,2) has the most windows in the final group, so round-robin emission
    # makes it the very last chain to finish
    LAST_CHAIN = (0, 2)

    with tile.TileContext(nc) as tc:
        with (
            tc.tile_pool(name="xp", bufs=1) as xp,
            tc.tile_pool(name="wp", bufs=1) as wp,
            tc.tile_pool(name="op", bufs=1) as op,
            tc.tile_pool(name="pp", bufs=1, space=bass.MemorySpace.PSUM) as pp,
        ):
            # x chunks on the sync (SP) HWDGE ring in consumption order
            xsb = xp.tile([128, NQ * BPC], mybir.dt.bfloat16, tag="xsb", name="xsb")
            for g0, g1 in ((0, 3), (3, 7), (7, 11), (11, NQ)):
                nc.sync.dma_start(
                    xsb[:, g0 * BPC : g1 * BPC],
                    x_d[:, g0 * BPC : g1 * BPC],
                )

            # W pieces on the scalar (ACT) ring, first-use order
            wsb = wp.tile([128, wtot], mybir.dt.bfloat16, tag="wsb", name="wsb")
            for lo, hi in wpieces:
                nc.scalar.dma_start(wsb[:, lo:hi], w_d[:, lo:hi])

            stgs = [
                op.tile([128, T], mybir.dt.bfloat16, tag=f"stg{s}", name=f"stg{s}")
                for s in range(NSC)
            ]

            # psum tags: groups alternate the two 4-bank halves
            psums = {}
            for gi, chains in enumerate(GROUPS):
                for ci, (s, I) in enumerate(chains):
                    psums[(s, I)] = pp.tile(
                        [128, 512],
                        mybir.dt.float32,
                        tag=f"ps{(gi % 2) * 4 + ci}",
                        name=f"ps_{s}_{I}",
                    )

            done = {s: 0 for s in range(NSC)}
            for s, I, q, w0, c0, c1, start, stop, chain_done in sched:
                nc.tensor.matmul(
                    psums[(s, I)][:, c0:c1],
                    xsb[:, q * BPC : (q + 1) * BPC],
                    wsb[:, bases[s] + w0 + c0 : bases[s] + w0 + c1],
                    start=start,
                    stop=stop,
                )
                if not chain_done:
                    continue
                stg = stgs[s]
                t0c = 512 * I
                done[s] += 1
                if (s, I) == LAST_CHAIN:
                    nc.vector.tensor_copy(
                        stg[:, t0c : t0c + 256], psums[(s, I)][:, 0:256]
                    )
                    nc.vector.tensor_copy(
                        stg[:, t0c + 256 : t0c + 512], psums[(s, I)][:, 256:512]
                    )
                else:
                    nc.vector.tensor_copy(stg[:, t0c : t0c + 512], psums[(s, I)][:])
                if s == 0:
                    # scale 1 brackets the schedule; store per-block pieces
                    if (s, I) == LAST_CHAIN:
                        # final chain: copy halves on two engines, stores on
                        # both rings, to shorten the tail
                        nc.sync.dma_start(
                            out_d[s][:, t0c : t0c + 256], stg[:, t0c : t0c + 256]
                        )
                        nc.scalar.dma_start(
                            out_d[s][:, t0c + 256 : t0c + 512],
                            stg[:, t0c + 256 : t0c + 512],
                        )
                    else:
                        nc.sync.dma_start(
                            out_d[s][:, t0c : t0c + 512], stg[:, t0c : t0c + 512]
                        )
                else:
                    if done[s] == NI:
                        # whole-scale store once the last block is staged
                        eng = nc.sync if OUT_ENG[s] == "sync" else nc.scalar
                        eng.dma_start(out_d[s], stg[:])

    nc.compile()
    return nc


def _program():
    global _NC_CACHE
    if _NC_CACHE is None:
        _NC_CACHE = _build_program()
    return _NC_CACHE


# ----------------------------------------------------------------- entry
def kernel(x: np.ndarray) -> np.ndarray:
    """x: [16, 2048, 64] float32 -> [16, 2048, 64, 5] float32"""
    global LAST_EXEC_NS
    import ml_dtypes
    from concourse.bass_utils import run_bass_kernel_spmd

    n, t, c = x.shape
    assert (t, n * c) == (T, B), (x.shape,)

    X = x.transpose(1, 0, 2).reshape(T, B).astype(np.float32)
    _, wbuf = _consts()
    in_maps = []
    for core in range(N_CORES):
        xc = X[:, core * BPC : (core + 1) * BPC]  # [2048, 128]
        xc = (
            xc.reshape(NQ, 128, BPC)
            .transpose(1, 0, 2)
            .reshape(128, NQ * BPC)
            .astype(ml_dtypes.bfloat16)
        )
        in_maps.append({"x": np.ascontiguousarray(xc), "w": wbuf})

    nc = _program()
    trace = bool(int(os.environ.get("CWT_TRACE", "0")))
    res = run_bass_kernel_spmd(nc, in_maps, list(range(N_CORES)), trace=trace)
    if trace:
        LAST_EXEC_NS = res.exec_time_ns
        globals()["LAST_RESULTS"] = res

    # per-core out: [5, 128, 2048] bf16 (b-local, t) -> Y [5, T, B] fp32
    Y = np.empty((NSC, T, B), np.float32)
    for core in range(N_CORES):
        o = np.asarray(res.results[core]["out"]).astype(np.float32)
        Y[:, :, core * BPC : (core + 1) * BPC] = o.transpose(0, 2, 1)
    return np.ascontiguousarray(
        Y.reshape(NSC, T, n, c).transpose(2, 1, 3, 0).astype(np.float32)
    )
